# revision 4
# baseline (speedup 1.0000x reference)
"""Causal self-attention (B=2, N=2048, D=2048, H=16, hd=128) on 8 Trainium2
NeuronCores — v3.

Tensor-parallel over heads (2 heads/core). v3 = v2 + full-K scores:
  - Per-head weight layout: each 128-row block of wqkT is one head's full
    head_dim, so q/k land on 128 partitions per head and the score matmul
    contracts K=128 in a single instruction (v1/v2 used two K=64 halves,
    which also kept the PE HAM throttled at 1.2 GHz through phase B).
  - RoPE rotate-half done with a PE permutation matmul (P constant) on the
    bf16 copy of the raw projection, then two DVE multiplies + one add.
  - Wide 2-bank exp, PE-accumulated softmax denominators,
    reciprocal_approx_fast, causal narrowing, pipelined score emission,
    double-buffered persist tiles, early woT prefetch (all from v2).
"""

import sys
import time

import ml_dtypes
import numpy as np

sys.path.insert(0, "/opt/trn_rl_repo")

import concourse.bacc as bacc  # noqa: E402
import concourse.bass as bass  # noqa: E402
import concourse.mybir as mybir  # noqa: E402
import concourse.tile as tile  # noqa: E402
from concourse import bass_utils  # noqa: E402

F32 = mybir.dt.float32
BF16 = mybir.dt.bfloat16

B, N, D = 2, 2048, 2048
H, HD = 16, 128
NC = 8
HPC = H // NC          # heads per core
BN = B * N             # 4096
NSH = BN // NC         # output rows per core
INNER = H * HD
ROPE_BASE = 10000.0

_CACHE = {}

LAST_EXEC_NS = None
LAST_RESULTS = None


def _build_program():
    nc = bacc.Bacc(
        "TRN2",
        target_bir_lowering=False,
        debug=False,
        enable_asserts=False,
        num_devices=NC,
    )
    xT = nc.dram_tensor("xT", [D, BN], BF16, kind="ExternalInput").ap()
    wqkT = nc.dram_tensor("wqkT", [D, 4 * HD], BF16, kind="ExternalInput").ap()
    wvT = nc.dram_tensor("wvT", [D, HPC * HD], BF16, kind="ExternalInput").ap()
    woT = nc.dram_tensor("woT", [INNER, D], BF16, kind="ExternalInput").ap()
    tabs = nc.dram_tensor("tabs", [4, HD, BN], BF16, kind="ExternalInput").ap()
    tri = nc.dram_tensor("tri", [128, 128], BF16, kind="ExternalInput").ap()
    rotp = nc.dram_tensor("rotp", [128, 128], BF16, kind="ExternalInput").ap()
    out = nc.dram_tensor("out", [NSH, D], F32, kind="ExternalOutput").ap()
    a2a_in = [
        nc.dram_tensor(f"a2a_in{h}", [NC, 128, 512], BF16).ap()
        for h in range(HPC)
    ]
    a2a_out = [
        nc.dram_tensor(f"a2a_out{h}", [NC, 128, 512], BF16).ap()
        for h in range(HPC)
    ]

    MUL = mybir.AluOpType.mult
    ADD = mybir.AluOpType.add
    SUB = mybir.AluOpType.subtract
    EXP = mybir.ActivationFunctionType.Exp

    with tile.TileContext(nc, num_cores=NC) as tc:
        with (
            tc.tile_pool(name="const", bufs=1) as constp,
            tc.tile_pool(name="wqk", bufs=1) as wqkp,
            tc.tile_pool(name="wv", bufs=1) as wvp,
            tc.tile_pool(name="persist", bufs=2) as persist,
            tc.tile_pool(name="wo", bufs=2) as wop,
        ):
            wqk_sb = wqkp.tile([128, 16, 512], BF16, name="wqk_sb")

            def emit_wqk_chunk(kc):
                nc.sync.dma_start(
                    out=wqk_sb[:, 4 * kc : 4 * kc + 4, :],
                    in_=wqkT.rearrange("(k p) m -> p k m", p=128)[
                        :, 4 * kc : 4 * kc + 4, :
                    ],
                )

            emit_wqk_chunk(0)
            tri_sb = constp.tile([128, 128], BF16, name="tri_sb")
            nc.sync.dma_start(out=tri_sb[:, :], in_=tri[:, :])
            rot_sb = constp.tile([128, 128], BF16, name="rot_sb")
            nc.sync.dma_start(out=rot_sb[:, :], in_=rotp[:, :])
            ones_col = constp.tile([128, 1], BF16, name="ones_col")
            nc.vector.memset(ones_col[:, :], 1.0)
            ones_row = constp.tile([1, 128], F32, name="ones_row")
            nc.vector.memset(ones_row[:, :], 1.0)
            wv_sb = wvp.tile([128, 16, 256], BF16, name="wv_sb")
            wo_tiles = {}

            def emit_wo_prefetch(dc):
                wt = wop.tile([128, 16, 512], BF16, tag="wo2", name=f"wo2_{dc}")
                nc.sync.dma_start(
                    out=wt[:, :, :],
                    in_=woT.rearrange("(k p) d -> p k d", p=128)[
                        :, :, 512 * dc : 512 * (dc + 1)
                    ],
                )
                wo_tiles[dc] = wt

            with (
                tc.tile_pool(name="xt", bufs=4) as xtp,
                tc.tile_pool(name="tab", bufs=3) as tabp,
                tc.tile_pool(name="rope", bufs=2) as ropep,
                tc.tile_pool(name="pt", bufs=3) as ptp,
                tc.tile_pool(name="small", bufs=2) as smallp,
                tc.tile_pool(name="ots", bufs=2) as otsp,
                tc.tile_pool(name="pq", bufs=2, space="PSUM") as pqp,
                tc.tile_pool(name="pov", bufs=2, space="PSUM") as povp,
                tc.tile_pool(name="psmall", bufs=1, space="PSUM") as psmallp,
            ):
                qkT_all = {}
                vT_all = {}
                for b in range(B):
                    qkT_all[b] = persist.tile(
                        [128, 4, N], BF16, tag="qkT", name=f"qkT_b{b}"
                    )
                    vT_all[b] = persist.tile(
                        [128, 16, HPC * HD], BF16, tag="vT", name=f"vT_b{b}"
                    )
                # ---------------- phase A: projection + RoPE (both batches) --
                for b in range(B):
                    qkT_sb = qkT_all[b]
                    vT_sb = vT_all[b]
                    for j in range(4):
                        n0 = b * N + 512 * j
                        first = b == 0 and j == 0
                        xh = []
                        for half in range(2):
                            t = xtp.tile(
                                [128, 8, 512], BF16, tag="xt",
                                name=f"xt_{b}_{j}_{half}",
                            )
                            for q4 in range(2):
                                nc.sync.dma_start(
                                    out=t[:, 4 * q4 : 4 * q4 + 4, :],
                                    in_=xT.rearrange("(k p) n -> p k n", p=128)[
                                        :,
                                        8 * half + 4 * q4 : 8 * half + 4 * q4 + 4,
                                        n0 : n0 + 512,
                                    ],
                                )
                                # interleave remaining weight chunks in the
                                # order the first projection block consumes them
                                if first and 2 * half + q4 < 3:
                                    emit_wqk_chunk(2 * half + q4 + 1)
                            xh.append(t)
                        if first:
                            nc.sync.dma_start(
                                out=wv_sb[:, :, :],
                                in_=wvT.rearrange("(k p) m -> p k m", p=128),
                            )
                        tab4 = tabp.tile(
                            [128, 4, 512], BF16, tag="tab", name=f"tab_{b}_{j}"
                        )
                        nc.sync.dma_start(
                            out=tab4[:, :, :],
                            in_=tabs.rearrange("t p n -> p t n")[
                                :, :, n0 : n0 + 512
                            ],
                        )
                        # qk2[:,0,:] = raw projection (PSUM); qk2[:,1,:] = rotated
                        # (PE permutation of the bf16 copy).  One pending rope
                        # per projection block keeps PE dense.
                        pending_rope = []

                        def emit_rope(mt, qk2):
                            # ci: 0/1 = scaled cos/sin (q heads), 2/3 = cos/sin (k)
                            ci = 0 if mt < 2 else 2
                            raw = ropep.tile(
                                [128, 512], BF16, tag="raw", name=f"raw_{b}_{j}_{mt}"
                            )
                            nc.scalar.copy(raw[:, :], qk2[:, 0, :])
                            nc.tensor.matmul(
                                qk2[:, 1, :], lhsT=rot_sb[:, :], rhs=raw[:, :],
                                start=True, stop=True,
                            )
                            t1 = ropep.tile([128, 512], BF16, tag="t1", name=f"t1_{b}_{j}_{mt}")
                            t2 = ropep.tile([128, 512], BF16, tag="t2", name=f"t2_{b}_{j}_{mt}")
                            nc.vector.tensor_tensor(t1[:, :], raw[:, :], tab4[:, ci, :], MUL)
                            nc.vector.tensor_tensor(t2[:, :], qk2[:, 1, :], tab4[:, ci + 1, :], MUL)
                            nc.vector.tensor_tensor(
                                qkT_sb[:, mt, 512 * j : 512 * (j + 1)], t1[:, :], t2[:, :], ADD
                            )

                        for mt in range(4):
                            qk2 = pqp.tile(
                                [128, 2, 512], F32, tag="st2",
                                name=f"qk2_{b}_{j}_{mt}",
                            )
                            for k in range(16):
                                nc.tensor.matmul(
                                    qk2[:, 0, :],
                                    lhsT=(wqk_sb[:, k, 128 * mt : 128 * mt + 128]),
                                    rhs=(xh[k // 8][:, k % 8, :]),
                                    start=(k == 0),
                                    stop=(k == 15),
                                )
                            if pending_rope:
                                emit_rope(*pending_rope.pop(0))
                            pending_rope.append((mt, qk2))
                        for mt in range(4):
                            pv = povp.tile(
                                [128, 256], F32, tag="ov", name=f"psV_{b}_{j}_{mt}"
                            )
                            for k in range(16):
                                nc.tensor.matmul(
                                    pv[:, :],
                                    lhsT=(xh[k // 8][:, k % 8, 128 * mt : 128 * mt + 128]),
                                    rhs=(wv_sb[:, k, :]),
                                    start=(k == 0),
                                    stop=(k == 15),
                                )
                            if pending_rope:
                                emit_rope(*pending_rope.pop(0))
                            nc.vector.tensor_copy(vT_sb[:, 4 * j + mt, :], pv[:, :])
                        while pending_rope:
                            emit_rope(*pending_rope.pop(0))
                    if b == 0:
                        emit_wo_prefetch(0)
                        emit_wo_prefetch(1)

                # ---------------- phase B: attention (h-major) -------------
                st2_store = {}
                unit_acc = {}

                def emit_scores(b, h, j, tt):
                    qkT_sb = qkT_all[b]
                    st2 = pqp.tile(
                        [128, 2, 512], F32, tag="st2",
                        name=f"st2_{b}_{h}_{j}_{tt}",
                    )
                    f0s = []
                    for u in (0, 1):
                        t = tt + u
                        f0 = max(0, 128 * t - 512 * j)
                        f0s.append(f0)
                        nc.tensor.matmul(
                            st2[:, u, f0:512],
                            lhsT=(qkT_sb[:, 2 + h, 128 * t : 128 * t + 128]),
                            rhs=(qkT_sb[:, h, 512 * j + f0 : 512 * (j + 1)]),
                            start=True,
                            stop=True,
                        )
                    st2_store[(b, h, j, tt)] = (st2, f0s)

                def emit_consume(b, h, j, tt):
                    vT_sb = vT_all[b]
                    nt = 4 * j + 4
                    if tt == 0:
                        unit_acc[(b, h, j)] = (
                            povp.tile([128, 512], F32, tag="ov", name=f"ov_{b}_{h}_{j}"),
                            ropep.tile([128, 512], BF16, tag="rsc", name=f"rsc_{b}_{h}_{j}"),
                        )
                    ov, rs_c = unit_acc[(b, h, j)]
                    st2, f0s = st2_store.pop((b, h, j, tt))
                    ws = min(f0s)
                    pt2 = ptp.tile(
                        [128, 2, 512], BF16, tag="pt", name=f"pt_{b}_{h}_{j}_{tt}"
                    )
                    nc.scalar.activation(
                        pt2[:, :, ws:512], st2[:, :, ws:512], EXP
                    )
                    for u in (0, 1):
                        t = tt + u
                        if t // 4 == j:
                            f0 = f0s[u]
                            nc.vector.tensor_tensor(
                                pt2[:, u, f0 : f0 + 128],
                                pt2[:, u, f0 : f0 + 128],
                                tri_sb[:, :],
                                MUL,
                            )
                    for u in (0, 1):
                        t = tt + u
                        f0 = f0s[u]
                        nc.tensor.matmul(
                            ov[:, f0:512],
                            lhsT=(vT_sb[:, t, 128 * h : 128 * h + 128]),
                            rhs=(pt2[:, u, f0:512]),
                            start=(t == 0),
                            stop=(t == nt - 1),
                        )
                        if t == 0:
                            nc.vector.tensor_copy(rs_c[:, :], pt2[:, 0, :])
                        else:
                            nc.vector.tensor_tensor(
                                rs_c[:, f0:512], rs_c[:, f0:512],
                                pt2[:, u, f0:512], ADD,
                            )

                def make_finalize(b, h, j):
                    ov, rs_c = unit_acc.pop((b, h, j))

                    def fin():
                        rsum = psmallp.tile([1, 512], F32, tag="rsum", name=f"rsum_{b}_{h}_{j}")
                        nc.tensor.matmul(
                            rsum[:, :], lhsT=ones_col[:, :], rhs=rs_c[:, :],
                            start=True, stop=True,
                        )
                        rsum_sb = smallp.tile([1, 512], F32, tag="rsum_sb", name=f"rsb_{b}_{h}_{j}")
                        nc.scalar.copy(rsum_sb[:, :], rsum[:, :])
                        binv = psmallp.tile([128, 512], F32, tag="binv", name=f"binv_{b}_{h}_{j}")
                        nc.tensor.matmul(
                            binv[:, :], lhsT=ones_row[:, :], rhs=rsum_sb[:, :],
                            start=True, stop=True,
                        )
                        rb = smallp.tile([128, 512], F32, tag="rb", name=f"rb_{b}_{h}_{j}")
                        nc.vector.reciprocal_approx_fast(rb[:, :], binv[:, :])
                        ot = otsp.tile([128, 512], BF16, tag="ot", name=f"ot_{b}_{h}_{j}")
                        nc.vector.tensor_tensor(ot[:, :], ov[:, :], rb[:, :], MUL)
                        nc.sync.dma_start(
                            out=a2a_in[h][4 * b + j, :, :], in_=ot[:, :]
                        )
                    return fin

                all_groups = []
                for h in range(HPC):
                    for b in range(B):
                        for j in range(4):
                            for tt in range(0, 4 * j + 4, 2):
                                all_groups.append((b, h, j, tt))
                pending_fin = None
                emit_scores(*all_groups[0])
                for gi, (b, h, j, tt) in enumerate(all_groups):
                    if gi + 1 < len(all_groups):
                        emit_scores(*all_groups[gi + 1])
                    if tt == 0 and pending_fin is not None:
                        pending_fin()
                        pending_fin = None
                    if (b, h, j, tt) == (0, 1, 0, 0):
                        # every h=0 finalize (both batches) has been emitted;
                        # reshard head 0 while head 1 attention still runs
                        nc.gpsimd.collective_compute(
                            "AllToAll",
                            mybir.AluOpType.bypass,
                            replica_groups=[list(range(NC))],
                            ins=[a2a_in[0].opt()],
                            outs=[a2a_out[0].opt()],
                        )
                    emit_consume(b, h, j, tt)
                    if tt == 4 * j + 2:  # last group of unit
                        pending_fin = make_finalize(b, h, j)
                pending_fin()
                pending_fin = None

            # ---------------- AllToAll reshard (head 1) --------------------
            nc.gpsimd.collective_compute(
                "AllToAll",
                mybir.AluOpType.bypass,
                replica_groups=[list(range(NC))],
                ins=[a2a_in[1].opt()],
                outs=[a2a_out[1].opt()],
            )

            # ---------------- phase C: o_proj ------------------------------
            # dc pairs; within a pair, all head-0 K-steps of both dc first so
            # the matmuls overlap the head-1 AllToAll
            with (
                tc.tile_pool(name="opin", bufs=1) as opinp,
                tc.tile_pool(name="outs", bufs=4) as outsp,
                tc.tile_pool(name="pc", bufs=8, space="PSUM") as pcp,
            ):
                opin = opinp.tile([128, 16, 512], BF16, name="opin")
                for h in range(HPC):
                    for r in range(NC):
                        nc.sync.dma_start(
                            out=opin[:, 2 * r + h, :], in_=a2a_out[h][r]
                        )
                def emit_outs(dc, pcs):
                    for ns in range(4):
                        ost = outsp.tile([128, 512], F32, tag="outs", name=f"os_{dc}_{ns}")
                        nc.vector.tensor_copy(ost[:, :], pcs[ns][:, :])
                        nc.sync.dma_start(
                            out=out[128 * ns : 128 * (ns + 1), 512 * dc : 512 * (dc + 1)],
                            in_=ost[:, :],
                        )

                for base in (0, 2):
                    pcs_all = {
                        dc: [
                            pcp.tile([128, 512], F32, tag="pc", name=f"pc_{dc}_{ns}")
                            for ns in range(4)
                        ]
                        for dc in (base, base + 1)
                    }

                    def emit_half(h, dc, start_h, stop_h):
                        wo2 = wo_tiles[dc]
                        for r in range(NC):
                            k = 2 * r + h
                            for ns in range(4):
                                nc.tensor.matmul(
                                    pcs_all[dc][ns][:, :],
                                    lhsT=(opin[:, k, 128 * ns : 128 * ns + 128]),
                                    rhs=(wo2[:, k, :]),
                                    start=(start_h and r == 0),
                                    stop=(stop_h and r == NC - 1),
                                )

                    emit_half(0, base, True, False)
                    emit_half(0, base + 1, True, False)
                    emit_half(1, base, False, True)
                    emit_outs(base, pcs_all[base])
                    emit_half(1, base + 1, False, True)
                    if base == 0:
                        emit_wo_prefetch(2)
                        emit_wo_prefetch(3)
                    emit_outs(base + 1, pcs_all[base + 1])
    nc.compile()
    return nc


def _host_prep(x, w_qkv, w_o):
    bf = ml_dtypes.bfloat16
    xT = np.ascontiguousarray(x.reshape(BN, D).T).astype(bf)
    woT = np.ascontiguousarray(np.asarray(w_o).T).astype(bf)

    inv_freq = 1.0 / (ROPE_BASE ** (np.arange(0, HD, 2, dtype=np.float32) / HD))
    ang = np.arange(N, dtype=np.float32)[:, None] * inv_freq[None, :]
    cos_h = np.cos(ang).T.astype(np.float32)      # [64, N]
    sin_h = np.sin(ang).T.astype(np.float32)      # [64, N] (magnitude)
    cos2 = np.concatenate([cos_h, cos_h], axis=0)  # [128, N]
    sin2 = np.concatenate([sin_h, sin_h], axis=0)
    cos_f = np.tile(cos2, (1, B))
    sin_f = np.tile(sin2, (1, B))
    scale = np.float32(1.0 / np.sqrt(HD))
    tabs = np.ascontiguousarray(
        np.stack([cos_f * scale, sin_f * scale, cos_f, sin_f], axis=0)
    ).astype(bf)

    # 128x128 lower-triangular block mask: key row p valid for query col c
    # within a diagonal 128x128 block iff p <= c
    p = np.arange(128)[:, None]
    c = np.arange(128)[None, :]
    tri = (p <= c).astype(bf)

    # rotate-half permutation: (P.T @ x)[i] = -x[i+64] (i<64), x[i-64] (i>=64)
    rotp = np.zeros((128, 128), dtype=np.float32)
    idx = np.arange(64)
    rotp[idx + 64, idx] = -1.0
    rotp[idx, idx + 64] = 1.0
    rotp = rotp.astype(bf)

    in_maps = []
    for core in range(NC):
        h0 = core * HPC
        rq = slice(h0 * HD, (h0 + HPC) * HD)
        rk = slice(INNER + h0 * HD, INNER + (h0 + HPC) * HD)
        rv = slice(2 * INNER + h0 * HD, 2 * INNER + (h0 + HPC) * HD)
        # per-head full-hd blocks: [q_h0, q_h1, k_h0, k_h1]
        wqkT = np.ascontiguousarray(
            np.concatenate([w_qkv[rq], w_qkv[rk]], axis=0).T
        ).astype(bf)
        wvT = np.ascontiguousarray(w_qkv[rv].T).astype(bf)
        in_maps.append(
            dict(xT=xT, wqkT=wqkT, wvT=wvT, woT=woT, tabs=tabs, tri=tri,
                 rotp=rotp)
        )
    return in_maps


def kernel(x, w_qkv, w_o, n_heads=None, head_dim=None, trace=False):
    global LAST_EXEC_NS, LAST_RESULTS
    x = np.asarray(x, dtype=np.float32)
    w_qkv = np.asarray(w_qkv, dtype=np.float32)
    w_o = np.asarray(w_o, dtype=np.float32)

    if "nc" not in _CACHE:
        _CACHE["nc"] = _build_program()
    nc = _CACHE["nc"]

    in_maps = _host_prep(x, w_qkv, w_o)
    res = None
    last_exc = None
    for attempt in range(4):
        try:
            res = bass_utils.run_bass_kernel_spmd(
                nc, in_maps, core_ids=list(range(NC)), trace=trace
            )
            break
        except Exception as e:  # transient compile_and_load / exec flakiness
            last_exc = e
            print(f"kernel attempt {attempt} failed: {e}", file=sys.stderr)
            time.sleep(5)
    if res is None:
        raise last_exc
    LAST_EXEC_NS = res.exec_time_ns
    LAST_RESULTS = res
    shards = [res.results[c]["out"] for c in range(NC)]
    full = np.concatenate(shards, axis=0).reshape(B, N, D).astype(np.float32)
    return full
